# revision 1
# baseline (speedup 1.0000x reference)
"""DrugGraphEmbedding (2x SAGEConv + sym-Laplacian features + mean-pool) on 8 trn2 cores.

Strategy: node-shard the 1024 graphs (128 graphs = 6144 nodes per core).
Aggregations (Laplacian smoothing, SAGE mean over incoming edges) run as
dma_gather of source rows + one-hot PE matmuls that scatter 128-edge chunks
into 128-destination PSUM windows.  Cross-core feature exchange uses two
AllGathers (lap features, hidden features).  Everything computes in fp16
with f32 PSUM accumulation; dense SAGE matmuls consume DMA-transposed
feature-major tiles.
"""

import numpy as np

B, S, D = 1024, 48, 256
GDIM = 512
N = B * S            # 49152
E = 4 * N            # 196608
NCORES = 8
NLOC = N // NCORES   # 6144
WIN = 128            # dst nodes per PSUM window
NWIN = NLOC // WIN   # 48
HALF = N // 2        # 24576 (int16 gather-table split)
GRP256 = 4           # chunks per gather call, 256-wide sources
GRP512 = 4           # chunks per gather call, 512-wide sources


def _pack_idx(idx_stream):
    """int16 stream -> [128, len/16] wrapped tile (16 partitions, replicated x8)."""
    L = len(idx_stream)
    assert L % 16 == 0
    w = idx_stream.reshape(L // 16, 16).T  # [16, L/16]
    return np.tile(w, (8, 1)).astype(np.int16)


def _build_streams(dst, src, wgt):
    """Pad edges into per-(core, window, src-half) groups with a shared
    chunks-per-window structure (SPMD: same program on every core).

    Returns (struct, per_core) where struct holds the compile-time shape info
    and per_core the data arrays."""
    half_of = (src >= HALF).astype(np.int64)
    gwin = dst // WIN  # global window id (core * NWIN + win)

    # group edges by (global window, half); order within a group is arbitrary
    order = np.lexsort((src, dst, half_of, gwin))
    dst_s, src_s = dst[order], src[order]
    wgt_s = wgt[order] if wgt is not None else None

    counts = np.zeros((NCORES * NWIN, 2), np.int64)
    np.add.at(counts, (gwin[order], half_of[order]), 1)
    flat_starts = np.concatenate([[0], np.cumsum(counts.reshape(-1))[:-1]]).reshape(
        NCORES * NWIN, 2
    )
    counts3 = counts.reshape(NCORES, NWIN, 2)
    cpw = np.ceil(counts3 / 128).astype(np.int64).max(axis=0)  # [NWIN, 2]
    cpw = np.maximum(cpw, 1)

    nch = [int(cpw[:, h].sum()) for h in (0, 1)]  # chunks per half-stream
    base = np.zeros((NWIN, 2), np.int64)
    base[1:, 0] = np.cumsum(cpw[:-1, 0])
    base[1:, 1] = np.cumsum(cpw[:-1, 1])

    per_core = []
    starts = {}
    for c in range(NCORES):
        for w in range(NWIN):
            for h in range(2):
                starts[(c, w, h)] = flat_starts[c * NWIN + w, h]

    for c in range(NCORES):
        idx_h, dstl_h, wgt_h = [], [], []
        for h in (0, 1):
            L = nch[h] * 128
            idx = np.zeros(L, np.int64)
            dl = np.full(L, -1.0, np.float16)  # pads never match iota 0..127
            wg = np.zeros(L, np.float16)
            for w in range(NWIN):
                n = counts3[c, w, h]
                s0 = starts[(c, w, h)]
                p0 = base[w, h] * 128
                idx[p0 : p0 + n] = src_s[s0 : s0 + n] - (HALF if h else 0)
                dl[p0 : p0 + n] = ((dst_s[s0 : s0 + n] % NLOC) % WIN).astype(np.float16)
                if wgt_s is not None:
                    wg[p0 : p0 + n] = wgt_s[s0 : s0 + n].astype(np.float16)
                else:
                    wg[p0 : p0 + n] = 1.0
                # pad slots: idx 0, weight 0 (dl 0 is fine since weight 0)
            assert idx.max(initial=0) < HALF
            idx_h.append(_pack_idx(idx.astype(np.int16)))
            dstl_h.append(np.ascontiguousarray(dl.reshape(nch[h], 128).T))
            wgt_h.append(np.ascontiguousarray(wg.reshape(nch[h], 128).T))
        per_core.append({"idx": idx_h, "dstl": dstl_h, "wgt": wgt_h})

    struct = {"cpw": cpw, "base": base, "nch": nch}
    return struct, per_core


def _host_prep(edge_index):
    row = np.asarray(edge_index[0], np.int64)
    col = np.asarray(edge_index[1], np.int64)

    deg = np.bincount(row, minlength=N).astype(np.float64)
    dinv = (deg > 0) / np.sqrt(np.maximum(deg, 1.0))
    cnt = np.bincount(col, minlength=N).astype(np.float64)
    cinv = 1.0 / np.maximum(cnt, 1.0)

    lap_w = -(dinv[row] * dinv[col])
    lap_struct, lap_pc = _build_streams(row, col, lap_w)     # dst=row, src=col
    conv_struct, conv_pc = _build_streams(col, row, None)    # dst=col, src=row

    cinv_tiles = []
    for c in range(NCORES):
        ci = cinv[c * NLOC : (c + 1) * NLOC].reshape(NWIN, WIN).T  # [128, NWIN]
        cinv_tiles.append(np.ascontiguousarray(ci).astype(np.float32))

    pool_dstl = np.zeros((128, NWIN), np.float16)
    for nt in range(NWIN):
        pool_dstl[:, nt] = ((nt * 128 + np.arange(128)) // S).astype(np.float16)

    iota = np.tile(np.arange(128, dtype=np.float16)[None, :], (128, 1))
    return {
        "lap": (lap_struct, lap_pc),
        "conv": (conv_struct, conv_pc),
        "cinv": cinv_tiles,
        "pool_dstl": pool_dstl,
        "iota": iota,
    }


def _build_program(lap_struct, conv_struct):
    import os

    import concourse.bass as bass
    import concourse.bacc as bacc
    import concourse.mybir as mybir
    from concourse.tile import TileContext

    phases = int(os.environ.get("KPHASES", "7"))
    wlim = int(os.environ.get("KWIN", str(NWIN)))

    fp16 = mybir.dt.float16
    f32 = mybir.dt.float32
    i16 = mybir.dt.int16

    nc = bacc.Bacc(
        "TRN2",
        target_bir_lowering=False,
        debug=False,
        num_devices=NCORES,
        dynamic_dma_scratch_size=24576,
        num_swdge_queues=4,
    )

    # ---- inputs -----------------------------------------------------------
    x16 = nc.dram_tensor("x16", [N, D], fp16, kind="ExternalInput")
    x16own = nc.dram_tensor("x16own", [NLOC, D], fp16, kind="ExternalInput")
    xT16 = nc.dram_tensor("xT16", [D, NLOC], fp16, kind="ExternalInput")
    iota_in = nc.dram_tensor("iota", [128, 128], fp16, kind="ExternalInput")
    ones_in = nc.dram_tensor("ones1", [1, 128], fp16, kind="ExternalInput")
    cinv_in = nc.dram_tensor("cinv", [128, NWIN], f32, kind="ExternalInput")
    pdstl_in = nc.dram_tensor("pool_dstl", [128, NWIN], fp16, kind="ExternalInput")

    wts_in = {}
    for nm in ("Wr1T", "Wl1T", "Wr2T", "Wl2T"):
        wts_in[nm] = nc.dram_tensor(nm, [GDIM, GDIM], fp16, kind="ExternalInput")
    b1_in = nc.dram_tensor("b1T", [1, GDIM], fp16, kind="ExternalInput")
    b2_in = nc.dram_tensor("b2T", [1, GDIM], fp16, kind="ExternalInput")

    lap_nch, conv_nch = lap_struct["nch"], conv_struct["nch"]
    lap_idx_in, conv_idx_in, lap_dstl_in, lap_w_in, conv_dstl_in = [], [], [], [], []
    for h in (0, 1):
        lap_idx_in.append(
            nc.dram_tensor(f"lap_idx{h}", [128, lap_nch[h] * 8], i16, kind="ExternalInput")
        )
        conv_idx_in.append(
            nc.dram_tensor(f"conv_idx{h}", [128, conv_nch[h] * 8], i16, kind="ExternalInput")
        )
        lap_dstl_in.append(
            nc.dram_tensor(f"lap_dstl{h}", [128, lap_nch[h]], fp16, kind="ExternalInput")
        )
        lap_w_in.append(
            nc.dram_tensor(f"lap_w{h}", [128, lap_nch[h]], fp16, kind="ExternalInput")
        )
        conv_dstl_in.append(
            nc.dram_tensor(f"conv_dstl{h}", [128, conv_nch[h]], fp16, kind="ExternalInput")
        )

    o_pool = nc.dram_tensor("o_pool", [128, GDIM], f32, kind="ExternalOutput")
    kdump = os.environ.get("KDUMP") == "1"
    dumps = {}
    if kdump:
        dumps["o_xc"] = nc.dram_tensor("o_xc", [NLOC, 2 * D], fp16, kind="ExternalOutput")
        dumps["o_m1"] = nc.dram_tensor("o_m1", [NLOC, GDIM], fp16, kind="ExternalOutput")
        dumps["o_h"] = nc.dram_tensor("o_h", [NLOC, GDIM], fp16, kind="ExternalOutput")
        dumps["o_m2"] = nc.dram_tensor("o_m2", [NLOC, GDIM], fp16, kind="ExternalOutput")

    # ---- internal DRAM ----------------------------------------------------
    xcomb16_own = nc.dram_tensor("xcomb16_own", [NLOC, 2 * D], fp16)
    xcomb16_full = nc.dram_tensor("xcomb16_full", [N, 2 * D], fp16, addr_space="Shared")
    h16_own = nc.dram_tensor("h16_own", [NLOC, GDIM], fp16)
    h16_full = nc.dram_tensor("h16_full", [N, GDIM], fp16, addr_space="Shared")
    m1_dram = nc.dram_tensor("m1_dram", [NLOC, GDIM], fp16)
    m2_dram = nc.dram_tensor("m2_dram", [NLOC, GDIM], fp16)

    RG = [list(range(NCORES))]

    with TileContext(nc) as tc:
        with (
            tc.tile_pool(name="const", bufs=1) as cpool,
            tc.tile_pool(name="msgs", bufs=16) as mpool,
            tc.tile_pool(name="asg", bufs=8) as apool,
            tc.tile_pool(name="tT", bufs=8) as tpool,
            tc.tile_pool(name="o16", bufs=4) as opool,
            tc.tile_pool(name="xw", bufs=3) as xwpool,
            tc.tile_pool(name="of32", bufs=1) as f32pool,
            tc.tile_pool(name="pagg", bufs=4, space="PSUM") as pagg,
            tc.tile_pool(name="pbig", bufs=3, space="PSUM") as pbig,
            tc.tile_pool(name="ppool", bufs=1, space="PSUM") as ppool,
        ):
            # ---- constants -----------------------------------------------
            iota = cpool.tile([128, 128], fp16, tag="iota")
            nc.sync.dma_start(out=iota[:], in_=iota_in[:])
            ones1 = cpool.tile([1, 128], fp16, tag="ones1")
            nc.sync.dma_start(out=ones1[:], in_=ones_in[:])
            cinv_t = cpool.tile([128, NWIN], f32, tag="cinv")
            nc.sync.dma_start(out=cinv_t[:], in_=cinv_in[:])
            pdstl = cpool.tile([128, NWIN], fp16, tag="pdstl")
            nc.sync.dma_start(out=pdstl[:], in_=pdstl_in[:])
            b1t = cpool.tile([1, GDIM], fp16, tag="b1")
            nc.sync.dma_start(out=b1t[:], in_=b1_in[:])
            b2t = cpool.tile([1, GDIM], fp16, tag="b2")
            nc.sync.dma_start(out=b2t[:], in_=b2_in[:])

            wt = {}
            for nm in ("Wr1T", "Wl1T", "Wr2T", "Wl2T"):
                t = cpool.tile([128, 4, GDIM], fp16, tag=nm)
                nc.sync.dma_start(
                    out=t[:], in_=wts_in[nm].ap().rearrange("(k p) f -> p k f", p=128)
                )
                wt[nm] = t

            lap_idx_t, conv_idx_t, lap_dstl_t, lap_w_t, conv_dstl_t = [], [], [], [], []
            for h in (0, 1):
                t = cpool.tile([128, lap_nch[h] * 8], i16, tag=f"lidx{h}")
                nc.sync.dma_start(out=t[:], in_=lap_idx_in[h][:])
                lap_idx_t.append(t)
                t = cpool.tile([128, conv_nch[h] * 8], i16, tag=f"cidx{h}")
                nc.sync.dma_start(out=t[:], in_=conv_idx_in[h][:])
                conv_idx_t.append(t)
                t = cpool.tile([128, lap_nch[h]], fp16, tag=f"ldstl{h}")
                nc.sync.dma_start(out=t[:], in_=lap_dstl_in[h][:])
                lap_dstl_t.append(t)
                t = cpool.tile([128, lap_nch[h]], fp16, tag=f"lw{h}")
                nc.sync.dma_start(out=t[:], in_=lap_w_in[h][:])
                lap_w_t.append(t)
                t = cpool.tile([128, conv_nch[h]], fp16, tag=f"cdstl{h}")
                nc.sync.dma_start(out=t[:], in_=conv_dstl_in[h][:])
                conv_dstl_t.append(t)

            # pool one-hot: [128, NWIN, 128]
            pool_asg = cpool.tile([128, NWIN, 128], fp16, tag="pasg")
            nc.vector.tensor_tensor(
                out=pool_asg[:],
                in0=pdstl[:].to_broadcast([128, NWIN, 128]),
                in1=iota[:, None, :].to_broadcast([128, NWIN, 128]),
                op=mybir.AluOpType.is_equal,
            )

            qctr = [0]

            class AggPlan:
                """Just-in-time gather + assign-build for one chunk-stream pass."""

                def __init__(self, struct, idx_tiles, tables, elem, grp,
                             dstl_tiles, w_tiles):
                    self.struct = struct
                    self.idx_tiles = idx_tiles
                    self.tables = tables
                    self.elem = elem
                    self.grp = grp
                    self.dstl_tiles = dstl_tiles
                    self.w_tiles = w_tiles
                    self.msgs = {}
                    self.asg = {}

                def _ensure(self, h, g):
                    if (h, g) in self.msgs:
                        return
                    c0 = g * self.grp
                    cn = min(self.grp, self.struct["nch"][h] - c0)
                    ni = cn * 128
                    tiles = []
                    for table in self.tables:
                        tile = mpool.tile([128, self.grp, self.elem], fp16, tag="msgs")
                        lo = table[0:HALF, :] if h == 0 else table[HALF:N, :]
                        nc.gpsimd.dma_gather(
                            out_ap=tile[:, 0:cn, :],
                            in_ap=lo,
                            idxs_ap=self.idx_tiles[h][:, c0 * 8 : (c0 + cn) * 8],
                            num_idxs=ni,
                            num_idxs_reg=ni,
                            elem_size=self.elem,
                            single_packet=False,
                            queue_num=qctr[0] % 4,
                        )
                        qctr[0] += 1
                        tiles.append(tile)
                    self.msgs[(h, g)] = tiles
                    t = apool.tile([128, self.grp, 128], fp16, tag="asg")
                    nc.vector.tensor_tensor(
                        out=t[:, 0:cn, :],
                        in0=self.dstl_tiles[h][:, c0 : c0 + cn].to_broadcast(
                            [128, cn, 128]
                        ),
                        in1=iota[:, None, :].to_broadcast([128, cn, 128]),
                        op=mybir.AluOpType.is_equal,
                    )
                    if self.w_tiles is not None:
                        nc.vector.tensor_tensor(
                            out=t[:, 0:cn, :],
                            in0=t[:, 0:cn, :],
                            in1=self.w_tiles[h][:, c0 : c0 + cn].to_broadcast(
                                [128, cn, 128]
                            ),
                            op=mybir.AluOpType.mult,
                        )
                    self.asg[(h, g)] = t

                def chunk(self, ci, h):
                    g, s = ci // self.grp, ci % self.grp
                    self._ensure(h, g)
                    return self.asg[(h, g)][:, s, :], [
                        m[:, s, :] for m in self.msgs[(h, g)]
                    ]

            def agg_windows(struct, plan, psum_shape, copy_out, n_mm_rhs=None):
                """Window loop: accumulate each window's chunks, then copy_out(w, ps)."""
                cpw, base = struct["cpw"], struct["base"]
                for w in range(wlim):
                    ps = pagg.tile(psum_shape, f32, tag="pagg")
                    total = int(cpw[w, 0] + cpw[w, 1])
                    k = 0
                    for h in (0, 1):
                        for j in range(int(cpw[w, h])):
                            ci = int(base[w, h]) + j
                            asg_ap, msg_aps = plan.chunk(ci, h)
                            nc.tensor.matmul(
                                out=ps[:],
                                lhsT=asg_ap,
                                rhs=msg_aps[0],
                                start=(k == 0),
                                stop=(k == total - 1),
                            )
                            k += 1
                    copy_out(w, ps)

            # ================= LAP phase ==================================
            with nc.named_scope("lap"):
                nc.sync.dma_start(out=xcomb16_own.ap()[:, 0:D], in_=x16own.ap())
                lap_plan = AggPlan(
                    lap_struct, lap_idx_t, [x16], D, GRP256, lap_dstl_t, lap_w_t
                )
                XWB = 8
                xw_holder = [None]

                def lap_out(w, ps):
                    if w % XWB == 0:
                        xw_holder[0] = xwpool.tile([128, XWB, D], fp16, tag="xw", name="xwb")
                        nc.sync.dma_start(
                            out=xw_holder[0][:],
                            in_=x16own.ap()[w * 128 : (w + XWB) * 128, :].rearrange(
                                "(b p) d -> p b d", p=128
                            ),
                        )
                    lt = opool.tile([128, D], fp16, tag="o16")
                    nc.vector.tensor_tensor(
                        out=lt[:],
                        in0=ps[:],
                        in1=xw_holder[0][:, w % XWB, :],
                        op=mybir.AluOpType.add,
                    )
                    nc.sync.dma_start(
                        out=xcomb16_own.ap()[w * 128 : (w + 1) * 128, D : 2 * D],
                        in_=lt[:],
                    )

                agg_windows(lap_struct, lap_plan, [128, D], lap_out)

                if phases >= 2:
                    nc.gpsimd.collective_compute(
                        "AllGather",
                        mybir.AluOpType.bypass,
                        replica_groups=RG,
                        ins=[xcomb16_own.ap().opt()],
                        outs=[xcomb16_full.ap().opt()],
                    )

            # ================= CONV1 aggregation ==========================
            if phases >= 3:
                with nc.named_scope("conv1_agg"):
                    c1_plan = AggPlan(
                        conv_struct, conv_idx_t, [xcomb16_full], 2 * D, GRP512,
                        conv_dstl_t, None,
                    )

                    def c1_out(w, ps):
                        mt = opool.tile([128, GDIM], fp16, tag="o16")
                        nc.vector.tensor_tensor(
                            out=mt[:],
                            in0=ps[:],
                            in1=cinv_t[:, w : w + 1].to_broadcast([128, GDIM]),
                            op=mybir.AluOpType.mult,
                        )
                        nc.sync.dma_start(
                            out=m1_dram[w * 128 : (w + 1) * 128, :], in_=mt[:]
                        )

                    agg_windows(conv_struct, c1_plan, [128, GDIM], c1_out)

            # ================= CONV1 dense ================================
            if phases >= 4:
                with nc.named_scope("conv1_dense"):
                    for nw in range(NLOC // 512):
                        r0 = nw * 512
                        lhs = {}
                        for name, dram, cof, nchk in (
                            ("xT", None, 0, 2),
                            ("lapT", xcomb16_own, D, 2),
                            ("m1T", m1_dram, 0, 4),
                        ):
                            tiles = []
                            for kk in range(nchk):
                                t = tpool.tile([128, 512], fp16, tag="tT")
                                if name == "xT":
                                    nc.sync.dma_start(
                                        out=t[:],
                                        in_=xT16[kk * 128 : (kk + 1) * 128, r0 : r0 + 512],
                                    )
                                else:
                                    nc.sync.dma_start_transpose(
                                        out=t[:],
                                        in_=dram[
                                            r0 : r0 + 512,
                                            cof + kk * 128 : cof + (kk + 1) * 128,
                                        ],
                                    )
                                tiles.append(t)
                            lhs[name] = tiles
                        for nt in range(4):
                            nsl = slice(nt * 128, (nt + 1) * 128)
                            ps = pbig.tile([128, GDIM], f32, tag="pbig")
                            mms = (
                                [("m1T", kk, "Wl1T", kk) for kk in range(4)]
                                + [("xT", kk, "Wr1T", kk) for kk in range(2)]
                                + [("lapT", kk, "Wr1T", kk + 2) for kk in range(2)]
                            )
                            for i, (ln, lk, wn, wk) in enumerate(mms):
                                nc.tensor.matmul(
                                    out=ps[:],
                                    lhsT=lhs[ln][lk][:, nsl],
                                    rhs=wt[wn][:, wk, :],
                                    start=(i == 0),
                                    stop=False,
                                )
                            nc.tensor.matmul(
                                out=ps[:], lhsT=ones1[:], rhs=b1t[:], start=False,
                                stop=True,
                            )
                            ht = opool.tile([128, GDIM], fp16, tag="o16")
                            nc.scalar.activation(
                                ht[:], ps[:], mybir.ActivationFunctionType.Gelu
                            )
                            nc.sync.dma_start(
                                out=h16_own[r0 + nt * 128 : r0 + (nt + 1) * 128, :],
                                in_=ht[:],
                            )

                    if phases >= 5:
                        nc.gpsimd.collective_compute(
                            "AllGather",
                            mybir.AluOpType.bypass,
                            replica_groups=RG,
                            ins=[h16_own.ap().opt()],
                            outs=[h16_full.ap().opt()],
                        )

            # ================= CONV2 aggregation ==========================
            if phases >= 6:
                with nc.named_scope("conv2_agg"):
                    c2_plan = AggPlan(
                        conv_struct, conv_idx_t, [h16_full], GDIM, GRP512,
                        conv_dstl_t, None,
                    )

                    def c2_out(w, ps):
                        mt = opool.tile([128, GDIM], fp16, tag="o16")
                        nc.vector.tensor_tensor(
                            out=mt[:],
                            in0=ps[:],
                            in1=cinv_t[:, w : w + 1].to_broadcast([128, GDIM]),
                            op=mybir.AluOpType.mult,
                        )
                        nc.sync.dma_start(
                            out=m2_dram[w * 128 : (w + 1) * 128, :], in_=mt[:]
                        )

                    agg_windows(conv_struct, c2_plan, [128, GDIM], c2_out)

            # ================= CONV2 dense + pool =========================
            if phases >= 7:
                with nc.named_scope("conv2_dense"):
                    ps_pool = ppool.tile([128, GDIM], f32, tag="ppool")
                    for nw in range(NLOC // 512):
                        r0 = nw * 512
                        lhs = {}
                        for name, dram in (("hT", h16_own), ("m2T", m2_dram)):
                            tiles = []
                            for kk in range(4):
                                t = tpool.tile([128, 512], fp16, tag="tT")
                                nc.sync.dma_start_transpose(
                                    out=t[:],
                                    in_=dram[r0 : r0 + 512, kk * 128 : (kk + 1) * 128],
                                )
                                tiles.append(t)
                            lhs[name] = tiles
                        for nt in range(4):
                            nsl = slice(nt * 128, (nt + 1) * 128)
                            ps = pbig.tile([128, GDIM], f32, tag="pbig")
                            for i, (ln, wn) in enumerate((("m2T", "Wl2T"), ("hT", "Wr2T"))):
                                for kk in range(4):
                                    nc.tensor.matmul(
                                        out=ps[:],
                                        lhsT=lhs[ln][kk][:, nsl],
                                        rhs=wt[wn][:, kk, :],
                                        start=(i == 0 and kk == 0),
                                        stop=False,
                                    )
                            nc.tensor.matmul(
                                out=ps[:], lhsT=ones1[:], rhs=b2t[:], start=False,
                                stop=True,
                            )
                            ot = opool.tile([128, GDIM], fp16, tag="o16")
                            nc.vector.tensor_copy(ot[:], ps[:])
                            ntg = nw * 4 + nt
                            nc.tensor.matmul(
                                out=ps_pool[:],
                                lhsT=pool_asg[:, ntg, :],
                                rhs=ot[:],
                                start=(ntg == 0),
                                stop=(ntg == NWIN - 1),
                            )
                    out_f = f32pool.tile([128, GDIM], f32, tag="of32")
                    nc.vector.tensor_scalar_mul(out_f[:], ps_pool[:], 1.0 / S)
                    nc.sync.dma_start(out=o_pool[:], in_=out_f[:])

            if phases < 7:
                dbg = f32pool.tile([128, GDIM], f32, tag="of32")
                nc.gpsimd.memset(dbg[:], 0.0)
                nc.sync.dma_start(out=o_pool[:], in_=dbg[:])
            if kdump:
                for nm, dram in (("o_xc", xcomb16_own), ("o_m1", m1_dram),
                                 ("o_h", h16_own), ("o_m2", m2_dram)):
                    nc.sync.dma_start(out=dumps[nm][:], in_=dram[:])

    nc.finalize()
    return nc


LAST_EXEC_NS = None
LAST_SCOPES = None


def _maybe_install_trace_hook():
    """Optional NTFF profiling (KTRACE=1): register the axon profile hook."""
    import sys
    import types

    try:
        from trn_agent_boot.trn_boot import _ntff_profile_via_ctypes

        hook = _ntff_profile_via_ctypes("/opt/axon/libaxon_pjrt.so")
        mod = types.ModuleType("antenv.axon_hooks")
        mod.get_axon_ntff_profile_hook = lambda: hook
        mod.set_axon_ntff_profile_hook = lambda h: None
        sys.modules["antenv.axon_hooks"] = mod
        return True
    except Exception:
        return False


def kernel(**inputs):
    import os

    from concourse.bass_utils import run_bass_kernel_spmd

    x = np.asarray(inputs["sub2gene_out"], np.float32).reshape(N, D)
    edge_index = np.asarray(inputs["edge_index"])
    W_l1 = np.asarray(inputs["W_l1"], np.float32)
    W_r1 = np.asarray(inputs["W_r1"], np.float32)
    b1 = np.asarray(inputs["b1"], np.float32)
    W_l2 = np.asarray(inputs["W_l2"], np.float32)
    W_r2 = np.asarray(inputs["W_r2"], np.float32)
    b2 = np.asarray(inputs["b2"], np.float32)

    prep = _host_prep(edge_index)
    lap_struct, lap_pc = prep["lap"]
    conv_struct, conv_pc = prep["conv"]

    nc = _build_program(lap_struct, conv_struct)

    x16 = x.astype(np.float16)
    wts = {
        "Wr1T": np.ascontiguousarray(W_r1.T).astype(np.float16),
        "Wl1T": np.ascontiguousarray(W_l1.T).astype(np.float16),
        "Wr2T": np.ascontiguousarray(W_r2.T).astype(np.float16),
        "Wl2T": np.ascontiguousarray(W_l2.T).astype(np.float16),
    }
    in_maps = []
    for c in range(NCORES):
        m = {
            "x16": x16,
            "x16own": x16[c * NLOC : (c + 1) * NLOC],
            "xT16": np.ascontiguousarray(x16[c * NLOC : (c + 1) * NLOC].T),
            "iota": prep["iota"],
            "ones1": np.ones((1, 128), np.float16),
            "cinv": prep["cinv"][c],
            "pool_dstl": prep["pool_dstl"],
            "b1T": b1.astype(np.float16)[None, :],
            "b2T": b2.astype(np.float16)[None, :],
            **wts,
        }
        for h in (0, 1):
            m[f"lap_idx{h}"] = lap_pc[c]["idx"][h]
            m[f"lap_dstl{h}"] = lap_pc[c]["dstl"][h]
            m[f"lap_w{h}"] = lap_pc[c]["wgt"][h]
            m[f"conv_idx{h}"] = conv_pc[c]["idx"][h]
            m[f"conv_dstl{h}"] = conv_pc[c]["dstl"][h]
        in_maps.append(m)

    trace = os.environ.get("KTRACE") == "1" and _maybe_install_trace_hook()
    res = run_bass_kernel_spmd(nc, in_maps, core_ids=list(range(NCORES)), trace=trace)
    global LAST_EXEC_NS, LAST_SCOPES, LAST_RESULTS, LAST_RES
    LAST_EXEC_NS = res.exec_time_ns
    LAST_SCOPES = res.per_core_scope_times
    LAST_RESULTS = res.results
    LAST_RES = res
    out = np.concatenate([res.results[c]["o_pool"] for c in range(NCORES)], axis=0)
    return out.astype(np.float32)

